# revision 5
# baseline (speedup 1.0000x reference)
"""Causal self-attention Trainium2 kernel (8 NeuronCores).

Sharding: core c = (batch b = c//2, head-group g = c%2). Each core computes
QKV projection for its 8 heads, causal attention, and a partial output
projection (row-slice of W_proj); host sums the two partials per batch.

All matmuls run as float32r (TF32-like, full PE rate); fp32 data is rounded
to f32r at tile-producer time (cast DMA / copy / activation outputs).
"""
import sys
sys.path.insert(0, '/opt/trn_rl_repo')
import numpy as np

B, T, C, H, D = 4, 2048, 1024, 16, 64
G = 2                 # head groups (tensor parallel)
HL = H // G           # 8 local heads
CP = HL * D           # 512
NP = HL // 2          # 4 head pairs
TB = 512              # t-block / q-block width
NTB = T // TB         # 4
NCH = C // 128        # 8 contraction chunks
NKC = T // 128        # 16 k chunks
NEG_CLAMP = -60.0

_PROG = None


def _build_program():
    import concourse.bass as bass
    from concourse import bacc
    import concourse.tile as tile
    from concourse import mybir

    f32 = mybir.dt.float32
    f32r = mybir.dt.float32r
    AF = mybir.ActivationFunctionType

    nc = bacc.Bacc()
    x_d = nc.dram_tensor("x_s", (T, C), f32, kind="ExternalInput")
    wqk_d = nc.dram_tensor("wqk", (C, 2 * CP), f32, kind="ExternalInput")
    wv_d = nc.dram_tensor("wv", (C, CP), f32, kind="ExternalInput")
    wp_d = nc.dram_tensor("wp", (CP, C), f32, kind="ExternalInput")
    mk_d = nc.dram_tensor("masks", (4, 128, TB), f32, kind="ExternalInput")
    id_d = nc.dram_tensor("ident", (128, 128), f32, kind="ExternalInput")
    on_d = nc.dram_tensor("onesd", (128, 64), f32, kind="ExternalInput")
    out_d = nc.dram_tensor("out_p", (T, C), f32, kind="ExternalOutput")
    k_d = nc.dram_tensor("k_p", (HL, T, D), f32, kind="ExternalOutput")
    v_d = nc.dram_tensor("v_p", (HL, T, D), f32, kind="ExternalOutput")

    with tile.TileContext(nc) as tc, \
         nc.allow_low_precision(reason="f32r compute validated against reference"):
        import contextlib
        with contextlib.ExitStack() as ctx:
            const = ctx.enter_context(tc.tile_pool(name="const", bufs=1))
            persist = ctx.enter_context(tc.tile_pool(name="persist", bufs=1))

            ident = const.tile([128, 128], f32r)
            nc.gpsimd.dma_start(out=ident, in_=id_d[:, :])
            ones = const.tile([128, 64], f32r)
            nc.gpsimd.dma_start(out=ones, in_=on_d[:, :])
            mask_sb = const.tile([128, 4 * TB], f32)
            for m in range(4):
                nc.sync.dma_start(out=mask_sb[:, m * TB:(m + 1) * TB], in_=mk_d[m, :, :])
            wp_sb = []
            for p in range(NP):
                t_ = persist.tile([128, C], f32r, name=f"wp{p}")
                nc.gpsimd.dma_start(out=t_, in_=wp_d[p * 128:(p + 1) * 128, :])
                wp_sb.append(t_)

            QtP, KtP = [], []
            for p in range(NP):
                QtP.append(persist.tile([128, T], f32r, name=f"qt{p}"))
                KtP.append(persist.tile([128, T], f32r, name=f"kt{p}"))
            Vp = []
            for h in range(HL):
                t_ = persist.tile([128, NKC * 65], f32r, name=f"vp{h}")
                nc.vector.tensor_copy(t_[:, 64::65], ones[:, 0:NKC])
                Vp.append(t_)

            # ---------------- Stage 1: QKV ----------------
            with tc.tile_pool(name="wvp", bufs=1) as wv_pool, \
                 tc.tile_pool(name="xs", bufs=5) as xs_pool, \
                 tc.tile_pool(name="xt", bufs=10) as xt_pool, \
                 tc.tile_pool(name="wqks", bufs=2) as wqk_pool, \
                 tc.tile_pool(name="kst", bufs=4) as kst_pool, \
                 tc.tile_pool(name="ps_tr", bufs=3, space="PSUM") as ps_tr, \
                 tc.tile_pool(name="ps_qk", bufs=2, space="PSUM") as ps_qk, \
                 tc.tile_pool(name="ps_v", bufs=2, space="PSUM") as ps_v:
                wv_sb = wv_pool.tile([128, NCH * CP], f32r)
                for cc in range(NCH):
                    nc.gpsimd.dma_start(out=wv_sb[:, cc * CP:(cc + 1) * CP],
                                        in_=wv_d[cc * 128:(cc + 1) * 128, :])
                for tb in range(NTB):
                    xts = []
                    xs_t = []
                    for j in range(4):
                        xst = xs_pool.tile([128, C], f32r, tag="xs", name=f"xs{tb}_{j}")
                        nc.gpsimd.dma_start(out=xst, in_=x_d[(tb * 4 + j) * 128:(tb * 4 + j + 1) * 128, :])
                        xs_t.append(xst)
                    for cc in range(NCH):
                        xtc = xt_pool.tile([128, TB], f32r, tag="xt", name=f"xt{tb}_{cc}")
                        for j in range(4):
                            tr = ps_tr.tile([128, 128], f32r, tag="tr", name=f"tr{tb}_{cc}_{j}")
                            nc.tensor.transpose(tr[:, :], xs_t[j][:, cc * 128:(cc + 1) * 128], ident)
                            if j % 2 == 0:
                                nc.scalar.copy(xtc[:, j * 128:(j + 1) * 128], tr[:, :])
                            else:
                                nc.vector.tensor_copy(xtc[:, j * 128:(j + 1) * 128], tr[:, :])
                        xts.append(xtc)
                    # Q,K col-blocks (transposed layout)
                    for cb in range(2 * NP):
                        wqkt = wqk_pool.tile([128, C], f32r, tag="wqk", name=f"wq{tb}_{cb}")
                        for cc in range(NCH):
                            nc.gpsimd.dma_start(
                                out=wqkt[:, cc * 128:(cc + 1) * 128],
                                in_=wqk_d[cc * 128:(cc + 1) * 128, cb * 128:(cb + 1) * 128])
                        pqk = ps_qk.tile([128, TB], f32, tag="pqk", name=f"pqk{tb}_{cb}")
                        for cc in range(NCH):
                            nc.tensor.matmul(pqk[:, :], wqkt[:, cc * 128:(cc + 1) * 128], xts[cc][:, :],
                                             start=(cc == 0), stop=(cc == NCH - 1))
                        dst = QtP[cb] if cb < NP else KtP[cb - NP]
                        nc.scalar.activation(out=dst[:, tb * TB:(tb + 1) * TB], in_=pqk[:, :], func=AF.Copy)
                    # V natural
                    for j in range(4):
                        tci = tb * 4 + j
                        pv = ps_v.tile([128, CP], f32, tag="pv", name=f"pv{tb}_{j}")
                        for cc in range(NCH):
                            nc.tensor.matmul(pv[:, :], xts[cc][:, j * 128:(j + 1) * 128],
                                             wv_sb[:, cc * CP:(cc + 1) * CP],
                                             start=(cc == 0), stop=(cc == NCH - 1))
                        for h in range(HL):
                            if h % 2 == 0:
                                nc.scalar.copy(Vp[h][:, tci * 65:tci * 65 + 64], pv[:, h * 64:(h + 1) * 64])
                            else:
                                nc.vector.tensor_copy(Vp[h][:, tci * 65:tci * 65 + 64], pv[:, h * 64:(h + 1) * 64])
                        for h in range(HL):
                            nc.sync.dma_start(out=v_d[h, tci * 128:(tci + 1) * 128, :],
                                              in_=Vp[h][:, tci * 65:tci * 65 + 64].bitcast(f32))
                # k natural outputs via PE transpose of Kt
                for p in range(NP):
                    for half in range(2):
                        rows = slice(half * 64, half * 64 + 64)
                        idsl = ident[rows, rows]
                        h = 2 * p + half
                        for tcn in range(NKC):
                            tr = ps_tr.tile([128, 64], f32r, tag="tr", name=f"ktr{p}_{half}_{tcn}")
                            nc.tensor.transpose(tr[:, :], KtP[p][rows, tcn * 128:(tcn + 1) * 128], idsl)
                            ks = kst_pool.tile([128, 64], f32, tag="ks", name=f"ks{p}_{half}_{tcn}")
                            if tcn % 2 == 0:
                                nc.scalar.copy(ks, tr[:, :])
                            else:
                                nc.vector.tensor_copy(ks, tr[:, :])
                            nc.sync.dma_start(out=k_d[h, tcn * 128:(tcn + 1) * 128, :], in_=ks)

            # ---------------- Stage 2+3: attention + projection ----------------
            with tc.tile_pool(name="ytp", bufs=1) as yt_pool, \
                 tc.tile_pool(name="ptp", bufs=4) as pt_pool, \
                 tc.tile_pool(name="work", bufs=2) as work, \
                 tc.tile_pool(name="ostg", bufs=3) as ostg_pool, \
                 tc.tile_pool(name="ps_s", bufs=3, space="PSUM") as ps_s, \
                 tc.tile_pool(name="ps_o", bufs=2, space="PSUM") as ps_o, \
                 tc.tile_pool(name="ps_rep", bufs=1, space="PSUM") as ps_rep, \
                 tc.tile_pool(name="ps_pr", bufs=2, space="PSUM") as ps_pr:
                YT = [yt_pool.tile([128, T], f32r, name=f"yt{p}") for p in range(NP)]
                for qb in range(NTB):
                    nkc = qb * 4 + 4
                    qsl = slice(qb * TB, (qb + 1) * TB)
                    for p in range(NP):
                        for half in range(2):
                            rows = slice(half * 64, half * 64 + 64)
                            h = 2 * p + half
                            o_ps = ps_o.tile([65, TB], f32, tag="o", name=f"o{qb}_{h}")
                            for kc in range(nkc):
                                s_ps = ps_s.tile([128, TB], f32, tag="s", name=f"s{qb}_{h}_{kc}")
                                nc.tensor.matmul(s_ps[:, :], KtP[p][rows, kc * 128:(kc + 1) * 128],
                                                 QtP[p][rows, qsl], start=True, stop=True)
                                dio = kc - qb * 4
                                if dio >= 0:
                                    nc.vector.tensor_add(out=s_ps[:, :], in0=s_ps[:, :],
                                                         in1=mask_sb[:, dio * TB:(dio + 1) * TB])
                                pt = pt_pool.tile([128, TB], f32r, tag="pt", name=f"pt{qb}_{h}_{kc}")
                                nc.scalar.activation(out=pt, in_=s_ps, func=AF.Exp)
                                nc.tensor.matmul(o_ps[:, :], Vp[h][:, kc * 65:(kc + 1) * 65], pt[:, :],
                                                 start=(kc == 0), stop=(kc == nkc - 1))
                            den = work.tile([65, TB], f32, tag="den", name=f"dn{qb}_{h}")
                            nc.scalar.activation(out=den[64:65, :], in_=o_ps[64:65, :], func=AF.Copy)
                            r = work.tile([65, TB], f32r, tag="r", name=f"r{qb}_{h}")
                            nc.vector.reciprocal(out=r[64:65, :], in_=den[64:65, :])
                            rep_ps = ps_rep.tile([64, TB], f32, tag="rep", name=f"rp{qb}_{h}")
                            nc.tensor.matmul(rep_ps[:, :], ones[64:65, :], r[64:65, :],
                                             start=True, stop=True)
                            o_sb = work.tile([64, TB], f32, tag="osb", name=f"ob{qb}_{h}")
                            nc.scalar.activation(out=o_sb, in_=o_ps[0:64, :], func=AF.Copy)
                            if half == 0:
                                nc.vector.tensor_mul(out=YT[p][0:64, qsl], in0=o_sb, in1=rep_ps[:, :])
                            else:
                                hop = work.tile([64, TB], f32r, tag="hop", name=f"hp{qb}_{h}")
                                nc.vector.tensor_mul(out=hop, in0=o_sb, in1=rep_ps[:, :])
                                nc.sync.dma_start(out=YT[p][64:128, qsl], in_=hop)
                    # projection for this q-block
                    for tt in range(4):
                        ti = qb * 4 + tt
                        for co in range(2):
                            pr = ps_pr.tile([128, 512], f32, tag="pr", name=f"pr{ti}_{co}")
                            for p in range(NP):
                                nc.tensor.matmul(pr[:, :], YT[p][:, ti * 128:(ti + 1) * 128],
                                                 wp_sb[p][:, co * 512:(co + 1) * 512],
                                                 start=(p == 0), stop=(p == NP - 1))
                            og = ostg_pool.tile([128, 512], f32, tag="og", name=f"og{ti}_{co}")
                            if co == 0:
                                nc.scalar.copy(og, pr[:, :])
                            else:
                                nc.vector.tensor_copy(og, pr[:, :])
                            nc.sync.dma_start(out=out_d[ti * 128:(ti + 1) * 128, co * 512:(co + 1) * 512],
                                              in_=og)
    nc.finalize()
    return nc


def get_program():
    global _PROG
    if _PROG is None:
        _PROG = _build_program()
    return _PROG


def _make_in_maps(x, mask, W_attn, W_proj):
    mask00 = np.asarray(mask[0, 0], dtype=np.float32)
    masks = np.stack([np.maximum(mask00[0:TB, m * 128:(m + 1) * 128].T, NEG_CLAMP)
                      for m in range(4)]).astype(np.float32)
    masks = np.ascontiguousarray(masks)
    ident = np.eye(128, dtype=np.float32)
    onesd = np.ones((128, 64), np.float32)
    in_maps = []
    for c in range(8):
        b, g = divmod(c, 2)
        wq = W_attn[:, g * CP:(g + 1) * CP] * np.float32(1.0 / np.sqrt(D))
        wk = W_attn[:, C + g * CP:C + (g + 1) * CP]
        wv = W_attn[:, 2 * C + g * CP:2 * C + (g + 1) * CP]
        in_maps.append({
            "x_s": np.ascontiguousarray(x[b]),
            "wqk": np.ascontiguousarray(np.concatenate([wq, wk], axis=1)),
            "wv": np.ascontiguousarray(wv),
            "wp": np.ascontiguousarray(W_proj[g * CP:(g + 1) * CP, :]),
            "masks": masks,
            "ident": ident,
            "onesd": onesd,
        })
    return in_maps


def _assemble(results, b_attn, b_proj, W_proj):
    out = np.empty((B, T, C), np.float32)
    k = np.empty((B, H, T, D), np.float32)
    v = np.empty((B, H, T, D), np.float32)
    bk = b_attn[C:2 * C]
    bv = b_attn[2 * C:3 * C]
    extra = b_proj.astype(np.float64) + bv.astype(np.float64) @ W_proj.astype(np.float64)
    for c in range(8):
        b, g = divmod(c, 2)
        r = results[c]
        if g == 0:
            out[b] = r["out_p"]
        else:
            out[b] += r["out_p"]
        for h in range(HL):
            gh = g * HL + h
            k[b, gh] = r["k_p"][h] + bk[gh * D:(gh + 1) * D][None, :]
            v[b, gh] = r["v_p"][h] + bv[gh * D:(gh + 1) * D][None, :]
    out += extra.astype(np.float32)[None, None, :]
    return out, k, v


def _reference_numpy(x, mask, W_attn, b_attn, W_proj, b_proj):
    x64 = x.astype(np.float64)
    qkv = x64 @ W_attn.astype(np.float64) + b_attn.astype(np.float64)
    q, kk, vv = np.split(qkv, 3, axis=2)
    q = q.reshape(B, T, H, D).transpose(0, 2, 1, 3)
    kk = kk.reshape(B, T, H, D).transpose(0, 2, 1, 3)
    vv = vv.reshape(B, T, H, D).transpose(0, 2, 1, 3)
    att = np.einsum('bhqd,bhkd->bhqk', q, kk) / np.sqrt(D)
    att = att + mask.astype(np.float64)
    att = att - att.max(axis=-1, keepdims=True)
    att = np.exp(att)
    att /= att.sum(axis=-1, keepdims=True)
    o = np.einsum('bhqk,bhkd->bhqd', att, vv)
    o = o.transpose(0, 2, 1, 3).reshape(B, T, C)
    o = o @ W_proj.astype(np.float64) + b_proj.astype(np.float64)
    return (o.astype(np.float32), kk.astype(np.float32), vv.astype(np.float32))


def kernel(x, mask, W_attn, b_attn, W_proj, b_proj):
    from concourse.bass_utils import run_bass_kernel_spmd
    x = np.asarray(x, np.float32)
    mask = np.asarray(mask, np.float32)
    W_attn = np.asarray(W_attn, np.float32)
    b_attn = np.asarray(b_attn, np.float32)
    W_proj = np.asarray(W_proj, np.float32)
    b_proj = np.asarray(b_proj, np.float32)

    causal = np.array_equal(
        np.asarray(mask[0, 0]),
        np.triu(np.full((T, T), -1e9, dtype=np.float32), k=1))
    if not causal or np.any(b_attn[0:C]):
        return _reference_numpy(x, mask, W_attn, b_attn, W_proj, b_proj)

    prog = get_program()
    in_maps = _make_in_maps(x, mask, W_attn, W_proj)
    res = run_bass_kernel_spmd(prog, in_maps, core_ids=list(range(8)))
    return _assemble(res.results, b_attn, b_proj, W_proj)


# revision 6
# speedup vs baseline: 1.3850x; 1.3850x over previous
"""Causal self-attention Trainium2 kernel (8 NeuronCores).

Sharding: core c = (batch b = c//2, head-group g = c%2). Each core computes
QKV projection for its 8 heads, causal attention, and a partial output
projection (row-slice of W_proj); host sums the two partials per batch.

All matmuls run as float32r (TF32-like, full PE rate); fp32 data is rounded
to f32r at tile-producer time (cast DMA / copy / activation outputs).
"""
import sys
sys.path.insert(0, '/opt/trn_rl_repo')
import numpy as np

B, T, C, H, D = 4, 2048, 1024, 16, 64
G = 2                 # head groups (tensor parallel)
HL = H // G           # 8 local heads
CP = HL * D           # 512
NP = HL // 2          # 4 head pairs
TB = 512              # t-block / q-block width
NTB = T // TB         # 4
NCH = C // 128        # 8 contraction chunks
NKC = T // 128        # 16 k chunks
NEG_CLAMP = -60.0

_PROG = None


def _build_program():
    import concourse.bass as bass
    from concourse import bacc
    import concourse.tile as tile
    from concourse import mybir

    f32 = mybir.dt.float32
    f32r = mybir.dt.float32r
    AF = mybir.ActivationFunctionType

    nc = bacc.Bacc()
    x_d = nc.dram_tensor("x_s", (T, C), f32r, kind="ExternalInput")
    wqk_d = nc.dram_tensor("wqk", (C, 2 * CP), f32r, kind="ExternalInput")
    wv_d = nc.dram_tensor("wv", (C, CP), f32r, kind="ExternalInput")
    wp_d = nc.dram_tensor("wp", (CP, C), f32r, kind="ExternalInput")
    mk_d = nc.dram_tensor("masks", (4, 128, TB), f32, kind="ExternalInput")
    id_d = nc.dram_tensor("ident", (128, 128), f32r, kind="ExternalInput")
    on_d = nc.dram_tensor("onesd", (128, 64), f32r, kind="ExternalInput")
    out_d = nc.dram_tensor("out_p", (T, C), f32, kind="ExternalOutput")
    k_d = nc.dram_tensor("k_p", (HL, T, D), f32, kind="ExternalOutput")
    v_d = nc.dram_tensor("v_p", (HL, T, D), f32, kind="ExternalOutput")

    with tile.TileContext(nc) as tc, \
         nc.allow_low_precision(reason="f32r compute validated against reference"):
        import contextlib
        with contextlib.ExitStack() as ctx:
            const = ctx.enter_context(tc.tile_pool(name="const", bufs=1))
            persist = ctx.enter_context(tc.tile_pool(name="persist", bufs=1))

            ident = const.tile([128, 128], f32r)
            nc.sync.dma_start(out=ident, in_=id_d[:, :])
            ones = const.tile([128, 64], f32r)
            nc.sync.dma_start(out=ones, in_=on_d[:, :])
            tri_sb = const.tile([128, 128], f32)
            nc.sync.dma_start(out=tri_sb, in_=mk_d[0, :, 0:128])
            wp_sb = []
            for p in range(NP):
                t_ = persist.tile([128, C], f32r, name=f"wp{p}")
                nc.sync.dma_start(out=t_, in_=wp_d[p * 128:(p + 1) * 128, :])
                wp_sb.append(t_)

            QtP, KtP = [], []
            for p in range(NP):
                QtP.append(persist.tile([128, T], f32r, name=f"qt{p}"))
                KtP.append(persist.tile([128, T], f32r, name=f"kt{p}"))
            Vp = []
            for h in range(HL):
                t_ = persist.tile([128, NKC * 65], f32r, name=f"vp{h}")
                nc.vector.tensor_copy(t_[:, 64::65], ones[:, 0:NKC])
                Vp.append(t_)

            # ---------------- Stage 1: QKV ----------------
            with tc.tile_pool(name="wvp", bufs=1) as wv_pool, \
                 tc.tile_pool(name="xs", bufs=5) as xs_pool, \
                 tc.tile_pool(name="xt", bufs=10) as xt_pool, \
                 tc.tile_pool(name="wqks", bufs=2) as wqk_pool, \
                 tc.tile_pool(name="kst", bufs=4) as kst_pool, \
                 tc.tile_pool(name="ps_tr", bufs=3, space="PSUM") as ps_tr, \
                 tc.tile_pool(name="ps_qk", bufs=2, space="PSUM") as ps_qk, \
                 tc.tile_pool(name="ps_v", bufs=2, space="PSUM") as ps_v:
                wv_sb = wv_pool.tile([128, NCH * CP], f32r)
                for cc in range(NCH):
                    nc.sync.dma_start(out=wv_sb[:, cc * CP:(cc + 1) * CP],
                                       in_=wv_d[cc * 128:(cc + 1) * 128, :])
                for tb in range(NTB):
                    xts = []
                    xs_t = []
                    for j in range(4):
                        xst = xs_pool.tile([128, C], f32r, tag="xs", name=f"xs{tb}_{j}")
                        nc.sync.dma_start(out=xst, in_=x_d[(tb * 4 + j) * 128:(tb * 4 + j + 1) * 128, :])
                        xs_t.append(xst)
                    for cc in range(NCH):
                        xtc = xt_pool.tile([128, TB], f32r, tag="xt", name=f"xt{tb}_{cc}")
                        for j in range(4):
                            tr = ps_tr.tile([128, 128], f32r, tag="tr", name=f"tr{tb}_{cc}_{j}")
                            nc.tensor.transpose(tr[:, :], xs_t[j][:, cc * 128:(cc + 1) * 128], ident)
                            if j % 2 == 0:
                                nc.scalar.copy(xtc[:, j * 128:(j + 1) * 128], tr[:, :])
                            else:
                                nc.vector.tensor_copy(xtc[:, j * 128:(j + 1) * 128], tr[:, :])
                        xts.append(xtc)
                    # Q,K col-blocks (transposed layout)
                    for cb in range(2 * NP):
                        wqkt = wqk_pool.tile([128, C], f32r, tag="wqk", name=f"wq{tb}_{cb}")
                        for cc in range(NCH):
                            nc.sync.dma_start(
                                out=wqkt[:, cc * 128:(cc + 1) * 128],
                                in_=wqk_d[cc * 128:(cc + 1) * 128, cb * 128:(cb + 1) * 128])
                        pqk = ps_qk.tile([128, TB], f32, tag="pqk", name=f"pqk{tb}_{cb}")
                        for cc in range(NCH):
                            nc.tensor.matmul(pqk[:, :], wqkt[:, cc * 128:(cc + 1) * 128], xts[cc][:, :],
                                             start=(cc == 0), stop=(cc == NCH - 1))
                        dst = QtP[cb] if cb < NP else KtP[cb - NP]
                        nc.scalar.activation(out=dst[:, tb * TB:(tb + 1) * TB], in_=pqk[:, :], func=AF.Copy)
                    # V natural
                    for j in range(4):
                        tci = tb * 4 + j
                        pv = ps_v.tile([128, CP], f32, tag="pv", name=f"pv{tb}_{j}")
                        for cc in range(NCH):
                            nc.tensor.matmul(pv[:, :], xts[cc][:, j * 128:(j + 1) * 128],
                                             wv_sb[:, cc * CP:(cc + 1) * CP],
                                             start=(cc == 0), stop=(cc == NCH - 1))
                        for h in range(HL):
                            if h % 2 == 0:
                                nc.scalar.copy(Vp[h][:, tci * 65:tci * 65 + 64], pv[:, h * 64:(h + 1) * 64])
                            else:
                                nc.vector.tensor_copy(Vp[h][:, tci * 65:tci * 65 + 64], pv[:, h * 64:(h + 1) * 64])
                    for h in range(HL):
                        src = Vp[h][:, tb * 260:(tb + 1) * 260].bitcast(f32).rearrange(
                            "p (c w) -> p c w", w=65)[:, :, 0:64]
                        dst = v_d[h, tb * TB:(tb + 1) * TB, :].rearrange("(c p) w -> p c w", p=128)
                        nc.sync.dma_start(out=dst, in_=src.bitcast(f32))
                # k natural outputs via PE transpose of Kt
                for p in range(NP):
                    for half in range(2):
                        rows = slice(half * 64, half * 64 + 64)
                        idsl = ident[rows, rows]
                        h = 2 * p + half
                        for grp in range(NKC // 4):
                            ks = kst_pool.tile([128, 256], f32, tag="ks", name=f"ks{p}_{half}_{grp}")
                            for q in range(4):
                                tcn = grp * 4 + q
                                tr = ps_tr.tile([128, 64], f32r, tag="tr", name=f"ktr{p}_{half}_{tcn}")
                                nc.tensor.transpose(tr[:, :], KtP[p][rows, tcn * 128:(tcn + 1) * 128], idsl)
                                if tcn % 2 == 0:
                                    nc.scalar.copy(ks[:, q * 64:(q + 1) * 64], tr[:, :])
                                else:
                                    nc.vector.tensor_copy(ks[:, q * 64:(q + 1) * 64], tr[:, :])
                            dstk = k_d[h, grp * 512:(grp + 1) * 512, :].rearrange("(c p) w -> p c w", p=128)
                            nc.sync.dma_start(out=dstk, in_=ks.rearrange("p (c w) -> p c w", w=64))

            # ---------------- Stage 2+3: attention + projection ----------------
            with tc.tile_pool(name="ytp", bufs=1) as yt_pool, \
                 tc.tile_pool(name="ptp", bufs=4) as pt_pool, \
                 tc.tile_pool(name="work", bufs=2) as work, \
                 tc.tile_pool(name="ostg", bufs=3) as ostg_pool, \
                 tc.tile_pool(name="ps_s", bufs=3, space="PSUM") as ps_s, \
                 tc.tile_pool(name="ps_o", bufs=2, space="PSUM") as ps_o, \
                 tc.tile_pool(name="ps_rep", bufs=1, space="PSUM") as ps_rep, \
                 tc.tile_pool(name="ps_pr", bufs=2, space="PSUM") as ps_pr:
                YT = [yt_pool.tile([128, T], f32r, name=f"yt{p}") for p in range(NP)]
                for qb in range(NTB):
                    nkc = qb * 4 + 4
                    qsl = slice(qb * TB, (qb + 1) * TB)
                    for p in range(NP):
                        for half in range(2):
                            rows = slice(half * 64, half * 64 + 64)
                            h = 2 * p + half
                            o_ps = ps_o.tile([65, TB], f32, tag="o", name=f"o{qb}_{h}")
                            for kc in range(nkc):
                                dio = kc - qb * 4
                                a0 = max(0, dio) * 128
                                csl = slice(a0, TB)
                                s_ps = ps_s.tile([128, TB], f32, tag="s", name=f"s{qb}_{h}_{kc}")
                                nc.tensor.matmul(s_ps[:, csl], KtP[p][rows, kc * 128:(kc + 1) * 128],
                                                 QtP[p][rows, qb * TB + a0:(qb + 1) * TB],
                                                 start=True, stop=True)
                                if dio >= 0:
                                    nc.vector.tensor_add(out=s_ps[:, a0:a0 + 128],
                                                         in0=s_ps[:, a0:a0 + 128], in1=tri_sb)
                                pt = pt_pool.tile([128, TB], f32r, tag="pt", name=f"pt{qb}_{h}_{kc}")
                                nc.scalar.activation(out=pt[:, csl], in_=s_ps[:, csl], func=AF.Exp)
                                nc.tensor.matmul(o_ps[:, csl], Vp[h][:, kc * 65:(kc + 1) * 65],
                                                 pt[:, csl], start=(kc == 0), stop=(kc == nkc - 1))
                            den = work.tile([65, TB], f32, tag="den", name=f"dn{qb}_{h}")
                            nc.vector.tensor_copy(den[64:65, :], o_ps[64:65, :])
                            r = work.tile([65, TB], f32r, tag="r", name=f"r{qb}_{h}")
                            nc.vector.reciprocal(out=r[64:65, :], in_=den[64:65, :])
                            rep_ps = ps_rep.tile([64, TB], f32, tag="rep", name=f"rp{qb}_{h}")
                            nc.tensor.matmul(rep_ps[:, :], ones[64:65, :], r[64:65, :],
                                             start=True, stop=True)
                            o_sb = work.tile([64, TB], f32, tag="osb", name=f"ob{qb}_{h}")
                            nc.vector.tensor_copy(o_sb, o_ps[0:64, :])
                            if half == 0:
                                nc.vector.tensor_mul(out=YT[p][0:64, qsl], in0=o_sb, in1=rep_ps[:, :])
                            else:
                                hop = work.tile([64, TB], f32r, tag="hop", name=f"hp{qb}_{h}")
                                nc.vector.tensor_mul(out=hop, in0=o_sb, in1=rep_ps[:, :])
                                nc.sync.dma_start(out=YT[p][64:128, qsl], in_=hop)
                    # projection for this q-block
                    for tt in range(4):
                        ti = qb * 4 + tt
                        for co in range(2):
                            pr = ps_pr.tile([128, 512], f32, tag="pr", name=f"pr{ti}_{co}")
                            for p in range(NP):
                                nc.tensor.matmul(pr[:, :], YT[p][:, ti * 128:(ti + 1) * 128],
                                                 wp_sb[p][:, co * 512:(co + 1) * 512],
                                                 start=(p == 0), stop=(p == NP - 1))
                            og = ostg_pool.tile([128, 512], f32, tag="og", name=f"og{ti}_{co}")
                            nc.vector.tensor_copy(og, pr[:, :])
                            nc.sync.dma_start(out=out_d[ti * 128:(ti + 1) * 128, co * 512:(co + 1) * 512],
                                              in_=og)
    nc.finalize()
    return nc


def get_program():
    global _PROG
    if _PROG is None:
        _PROG = _build_program()
    return _PROG


def _make_in_maps(x, mask, W_attn, W_proj):
    mask00 = np.asarray(mask[0, 0], dtype=np.float32)
    masks = np.stack([np.maximum(mask00[0:TB, m * 128:(m + 1) * 128].T, NEG_CLAMP)
                      for m in range(4)]).astype(np.float32)
    masks = np.ascontiguousarray(masks)
    ident = np.eye(128, dtype=np.float32)
    onesd = np.ones((128, 64), np.float32)
    in_maps = []
    for c in range(8):
        b, g = divmod(c, 2)
        wq = W_attn[:, g * CP:(g + 1) * CP] * np.float32(1.0 / np.sqrt(D))
        wk = W_attn[:, C + g * CP:C + (g + 1) * CP]
        wv = W_attn[:, 2 * C + g * CP:2 * C + (g + 1) * CP]
        in_maps.append({
            "x_s": np.ascontiguousarray(x[b]),
            "wqk": np.ascontiguousarray(np.concatenate([wq, wk], axis=1)),
            "wv": np.ascontiguousarray(wv),
            "wp": np.ascontiguousarray(W_proj[g * CP:(g + 1) * CP, :]),
            "masks": masks,
            "ident": ident,
            "onesd": onesd,
        })
    return in_maps


def _assemble(results, b_attn, b_proj, W_proj):
    out = np.empty((B, T, C), np.float32)
    k = np.empty((B, H, T, D), np.float32)
    v = np.empty((B, H, T, D), np.float32)
    bk = b_attn[C:2 * C]
    bv = b_attn[2 * C:3 * C]
    extra = b_proj.astype(np.float64) + bv.astype(np.float64) @ W_proj.astype(np.float64)
    for c in range(8):
        b, g = divmod(c, 2)
        r = results[c]
        if g == 0:
            out[b] = r["out_p"]
        else:
            out[b] += r["out_p"]
        for h in range(HL):
            gh = g * HL + h
            k[b, gh] = r["k_p"][h] + bk[gh * D:(gh + 1) * D][None, :]
            v[b, gh] = r["v_p"][h] + bv[gh * D:(gh + 1) * D][None, :]
    out += extra.astype(np.float32)[None, None, :]
    return out, k, v


def _reference_numpy(x, mask, W_attn, b_attn, W_proj, b_proj):
    x64 = x.astype(np.float64)
    qkv = x64 @ W_attn.astype(np.float64) + b_attn.astype(np.float64)
    q, kk, vv = np.split(qkv, 3, axis=2)
    q = q.reshape(B, T, H, D).transpose(0, 2, 1, 3)
    kk = kk.reshape(B, T, H, D).transpose(0, 2, 1, 3)
    vv = vv.reshape(B, T, H, D).transpose(0, 2, 1, 3)
    att = np.einsum('bhqd,bhkd->bhqk', q, kk) / np.sqrt(D)
    att = att + mask.astype(np.float64)
    att = att - att.max(axis=-1, keepdims=True)
    att = np.exp(att)
    att /= att.sum(axis=-1, keepdims=True)
    o = np.einsum('bhqk,bhkd->bhqd', att, vv)
    o = o.transpose(0, 2, 1, 3).reshape(B, T, C)
    o = o @ W_proj.astype(np.float64) + b_proj.astype(np.float64)
    return (o.astype(np.float32), kk.astype(np.float32), vv.astype(np.float32))


def kernel(x, mask, W_attn, b_attn, W_proj, b_proj):
    from concourse.bass_utils import run_bass_kernel_spmd
    x = np.asarray(x, np.float32)
    mask = np.asarray(mask, np.float32)
    W_attn = np.asarray(W_attn, np.float32)
    b_attn = np.asarray(b_attn, np.float32)
    W_proj = np.asarray(W_proj, np.float32)
    b_proj = np.asarray(b_proj, np.float32)

    causal = np.array_equal(
        np.asarray(mask[0, 0]),
        np.triu(np.full((T, T), -1e9, dtype=np.float32), k=1))
    if not causal or np.any(b_attn[0:C]):
        return _reference_numpy(x, mask, W_attn, b_attn, W_proj, b_proj)

    prog = get_program()
    in_maps = _make_in_maps(x, mask, W_attn, W_proj)
    res = run_bass_kernel_spmd(prog, in_maps, core_ids=list(range(8)))
    return _assemble(res.results, b_attn, b_proj, W_proj)


# revision 7
# speedup vs baseline: 1.3906x; 1.0041x over previous
"""Causal self-attention Trainium2 kernel (8 NeuronCores).

Sharding: core c = (batch b = c//2, head-group g = c%2). Each core computes
QKV projection for its 8 heads, causal attention, and a partial output
projection (row-slice of W_proj); host sums the two partials per batch.

All matmuls run as float32r (TF32-like, full PE rate); fp32 data is rounded
to f32r at tile-producer time (cast DMA / copy / activation outputs).
"""
import sys
sys.path.insert(0, '/opt/trn_rl_repo')
import numpy as np

B, T, C, H, D = 4, 2048, 1024, 16, 64
G = 2                 # head groups (tensor parallel)
HL = H // G           # 8 local heads
CP = HL * D           # 512
NP = HL // 2          # 4 head pairs
TB = 512              # t-block / q-block width
NTB = T // TB         # 4
NCH = C // 128        # 8 contraction chunks
NKC = T // 128        # 16 k chunks
NEG_CLAMP = -60.0

_PROG = None


def _build_program():
    import concourse.bass as bass
    from concourse import bacc
    import concourse.tile as tile
    from concourse import mybir

    f32 = mybir.dt.float32
    f32r = mybir.dt.float32r
    AF = mybir.ActivationFunctionType

    nc = bacc.Bacc()
    x_d = nc.dram_tensor("x_s", (T, C), f32r, kind="ExternalInput")
    wqk_d = nc.dram_tensor("wqk", (C, 2 * CP), f32r, kind="ExternalInput")
    wv_d = nc.dram_tensor("wv", (C, CP), f32r, kind="ExternalInput")
    wp_d = nc.dram_tensor("wp", (CP, C), f32r, kind="ExternalInput")
    mk_d = nc.dram_tensor("masks", (4, 128, TB), f32, kind="ExternalInput")
    id_d = nc.dram_tensor("ident", (128, 128), f32r, kind="ExternalInput")
    on_d = nc.dram_tensor("onesd", (128, 64), f32r, kind="ExternalInput")
    out_d = nc.dram_tensor("out_p", (T, C), f32, kind="ExternalOutput")
    k_d = nc.dram_tensor("k_p", (HL, T, D), f32, kind="ExternalOutput")
    v_d = nc.dram_tensor("v_p", (HL, T, D), f32, kind="ExternalOutput")

    with tile.TileContext(nc) as tc, \
         nc.allow_low_precision(reason="f32r compute validated against reference"):
        import contextlib
        with contextlib.ExitStack() as ctx:
            const = ctx.enter_context(tc.tile_pool(name="const", bufs=1))
            persist = ctx.enter_context(tc.tile_pool(name="persist", bufs=1))

            ident = const.tile([128, 128], f32r)
            nc.sync.dma_start(out=ident, in_=id_d[:, :])
            ones = const.tile([128, 64], f32r)
            nc.sync.dma_start(out=ones, in_=on_d[:, :])
            tri_sb = const.tile([128, 128], f32)
            nc.sync.dma_start(out=tri_sb, in_=mk_d[0, :, 0:128])
            wp_sb = []
            for p in range(NP):
                t_ = persist.tile([128, C], f32r, name=f"wp{p}")
                nc.sync.dma_start(out=t_, in_=wp_d[p * 128:(p + 1) * 128, :])
                wp_sb.append(t_)

            QtP, KtP = [], []
            for p in range(NP):
                QtP.append(persist.tile([128, T], f32r, name=f"qt{p}"))
                KtP.append(persist.tile([128, T], f32r, name=f"kt{p}"))
            Vp = []
            for h in range(HL):
                t_ = persist.tile([128, NKC * 65], f32r, name=f"vp{h}")
                nc.vector.tensor_copy(t_[:, 64::65], ones[:, 0:NKC])
                Vp.append(t_)

            # ---------------- Stage 1: QKV ----------------
            with tc.tile_pool(name="wvp", bufs=1) as wv_pool, \
                 tc.tile_pool(name="xs", bufs=6) as xs_pool, \
                 tc.tile_pool(name="xt", bufs=10) as xt_pool, \
                 tc.tile_pool(name="wqks", bufs=2) as wqk_pool, \
                 tc.tile_pool(name="kst", bufs=4) as kst_pool, \
                 tc.tile_pool(name="ps_tr", bufs=4, space="PSUM") as ps_tr, \
                 tc.tile_pool(name="ps_qk", bufs=2, space="PSUM") as ps_qk, \
                 tc.tile_pool(name="ps_v", bufs=2, space="PSUM") as ps_v:
                wv_sb = wv_pool.tile([128, NCH * CP], f32r)
                for cc in range(NCH):
                    nc.sync.dma_start(out=wv_sb[:, cc * CP:(cc + 1) * CP],
                                       in_=wv_d[cc * 128:(cc + 1) * 128, :])
                for tb in range(NTB):
                    xts = []
                    xs_t = []
                    for j in range(4):
                        xst = xs_pool.tile([128, C], f32r, tag="xs", name=f"xs{tb}_{j}")
                        nc.sync.dma_start(out=xst, in_=x_d[(tb * 4 + j) * 128:(tb * 4 + j + 1) * 128, :])
                        xs_t.append(xst)
                    for cc in range(NCH):
                        xtc = xt_pool.tile([128, TB], f32r, tag="xt", name=f"xt{tb}_{cc}")
                        for j in range(4):
                            tr = ps_tr.tile([128, 128], f32r, tag="tr", name=f"tr{tb}_{cc}_{j}")
                            nc.tensor.transpose(tr[:, :], xs_t[j][:, cc * 128:(cc + 1) * 128], ident)
                            nc.vector.tensor_copy(xtc[:, j * 128:(j + 1) * 128], tr[:, :])
                        xts.append(xtc)
                    # Q,K col-blocks (transposed layout)
                    for cb in range(2 * NP):
                        wqkt = wqk_pool.tile([128, C], f32r, tag="wqk", name=f"wq{tb}_{cb}")
                        nc.sync.dma_start(
                            out=wqkt.rearrange("p (c w) -> p c w", w=128),
                            in_=wqk_d[:, cb * 128:(cb + 1) * 128].rearrange("(c p) w -> p c w", p=128))
                        pqk = ps_qk.tile([128, TB], f32, tag="pqk", name=f"pqk{tb}_{cb}")
                        for cc in range(NCH):
                            nc.tensor.matmul(pqk[:, :], wqkt[:, cc * 128:(cc + 1) * 128], xts[cc][:, :],
                                             start=(cc == 0), stop=(cc == NCH - 1))
                        dst = QtP[cb] if cb < NP else KtP[cb - NP]
                        nc.scalar.activation(out=dst[:, tb * TB:(tb + 1) * TB], in_=pqk[:, :], func=AF.Copy)
                    # V natural
                    for j in range(4):
                        tci = tb * 4 + j
                        pv = ps_v.tile([128, CP], f32, tag="pv", name=f"pv{tb}_{j}")
                        for cc in range(NCH):
                            nc.tensor.matmul(pv[:, :], xts[cc][:, j * 128:(j + 1) * 128],
                                             wv_sb[:, cc * CP:(cc + 1) * CP],
                                             start=(cc == 0), stop=(cc == NCH - 1))
                        for h in range(HL):
                            if h % 4 != 0:
                                nc.vector.tensor_copy(Vp[h][:, tci * 65:tci * 65 + 64], pv[:, h * 64:(h + 1) * 64])
                            else:
                                nc.scalar.copy(Vp[h][:, tci * 65:tci * 65 + 64], pv[:, h * 64:(h + 1) * 64])
                    for h in range(HL):
                        src = Vp[h][:, tb * 260:(tb + 1) * 260].bitcast(f32).rearrange(
                            "p (c w) -> p c w", w=65)[:, :, 0:64]
                        dst = v_d[h, tb * TB:(tb + 1) * TB, :].rearrange("(c p) w -> p c w", p=128)
                        nc.gpsimd.dma_start(out=dst, in_=src.bitcast(f32))
                # k natural outputs via PE transpose of Kt
                for p in range(NP):
                    for half in range(2):
                        rows = slice(half * 64, half * 64 + 64)
                        idsl = ident[rows, rows]
                        h = 2 * p + half
                        for grp in range(NKC // 4):
                            ks = kst_pool.tile([128, 256], f32, tag="ks", name=f"ks{p}_{half}_{grp}")
                            for q in range(4):
                                tcn = grp * 4 + q
                                tr = ps_tr.tile([128, 64], f32r, tag="tr", name=f"ktr{p}_{half}_{tcn}")
                                nc.tensor.transpose(tr[:, :], KtP[p][rows, tcn * 128:(tcn + 1) * 128], idsl)
                                if tcn % 2 == 0:
                                    nc.scalar.copy(ks[:, q * 64:(q + 1) * 64], tr[:, :])
                                else:
                                    nc.vector.tensor_copy(ks[:, q * 64:(q + 1) * 64], tr[:, :])
                            dstk = k_d[h, grp * 512:(grp + 1) * 512, :].rearrange("(c p) w -> p c w", p=128)
                            nc.gpsimd.dma_start(out=dstk, in_=ks.rearrange("p (c w) -> p c w", w=64))

            # ---------------- Stage 2+3: attention + projection ----------------
            with tc.tile_pool(name="ytp", bufs=1) as yt_pool, \
                 tc.tile_pool(name="ptp", bufs=6) as pt_pool, \
                 tc.tile_pool(name="work", bufs=2) as work, \
                 tc.tile_pool(name="ostg", bufs=3) as ostg_pool, \
                 tc.tile_pool(name="ps_s", bufs=3, space="PSUM") as ps_s, \
                 tc.tile_pool(name="ps_o", bufs=2, space="PSUM") as ps_o, \
                 tc.tile_pool(name="ps_rep", bufs=1, space="PSUM") as ps_rep, \
                 tc.tile_pool(name="ps_pr", bufs=2, space="PSUM") as ps_pr:
                YT = [yt_pool.tile([128, T], f32r, name=f"yt{p}") for p in range(NP)]
                for qb in range(NTB):
                    nkc = qb * 4 + 4
                    qsl = slice(qb * TB, (qb + 1) * TB)
                    for p in range(NP):
                        for half in range(2):
                            rows = slice(half * 64, half * 64 + 64)
                            h = 2 * p + half
                            o_ps = ps_o.tile([65, TB], f32, tag="o", name=f"o{qb}_{h}")
                            for kc in range(nkc):
                                dio = kc - qb * 4
                                a0 = max(0, dio) * 128
                                csl = slice(a0, TB)
                                s_ps = ps_s.tile([128, TB], f32, tag="s", name=f"s{qb}_{h}_{kc}")
                                nc.tensor.matmul(s_ps[:, csl], KtP[p][rows, kc * 128:(kc + 1) * 128],
                                                 QtP[p][rows, qb * TB + a0:(qb + 1) * TB],
                                                 start=True, stop=True)
                                if dio >= 0:
                                    nc.vector.tensor_add(out=s_ps[:, a0:a0 + 128],
                                                         in0=s_ps[:, a0:a0 + 128], in1=tri_sb)
                                pt = pt_pool.tile([128, TB], f32r, tag="pt", name=f"pt{qb}_{h}_{kc}")
                                nc.scalar.activation(out=pt[:, csl], in_=s_ps[:, csl], func=AF.Exp)
                                nc.tensor.matmul(o_ps[:, csl], Vp[h][:, kc * 65:(kc + 1) * 65],
                                                 pt[:, csl], start=(kc == 0), stop=(kc == nkc - 1))
                            den = work.tile([65, TB], f32, tag="den", name=f"dn{qb}_{h}")
                            nc.vector.tensor_copy(den[64:65, :], o_ps[64:65, :])
                            r = work.tile([65, TB], f32r, tag="r", name=f"r{qb}_{h}")
                            nc.vector.reciprocal(out=r[64:65, :], in_=den[64:65, :])
                            rep_ps = ps_rep.tile([64, TB], f32, tag="rep", name=f"rp{qb}_{h}")
                            nc.tensor.matmul(rep_ps[:, :], ones[64:65, :], r[64:65, :],
                                             start=True, stop=True)
                            o_sb = work.tile([64, TB], f32, tag="osb", name=f"ob{qb}_{h}")
                            nc.vector.tensor_copy(o_sb, o_ps[0:64, :])
                            if half == 0:
                                nc.vector.tensor_mul(out=YT[p][0:64, qsl], in0=o_sb, in1=rep_ps[:, :])
                            else:
                                hop = work.tile([64, TB], f32r, tag="hop", name=f"hp{qb}_{h}")
                                nc.vector.tensor_mul(out=hop, in0=o_sb, in1=rep_ps[:, :])
                                nc.sync.dma_start(out=YT[p][64:128, qsl], in_=hop)
                    # projection for this q-block
                    for tt in range(4):
                        ti = qb * 4 + tt
                        for co in range(2):
                            pr = ps_pr.tile([128, 512], f32, tag="pr", name=f"pr{ti}_{co}")
                            for p in range(NP):
                                nc.tensor.matmul(pr[:, :], YT[p][:, ti * 128:(ti + 1) * 128],
                                                 wp_sb[p][:, co * 512:(co + 1) * 512],
                                                 start=(p == 0), stop=(p == NP - 1))
                            og = ostg_pool.tile([128, 512], f32, tag="og", name=f"og{ti}_{co}")
                            nc.vector.tensor_copy(og, pr[:, :])
                            nc.sync.dma_start(out=out_d[ti * 128:(ti + 1) * 128, co * 512:(co + 1) * 512],
                                              in_=og)
    nc.finalize()
    return nc


def get_program():
    global _PROG
    if _PROG is None:
        _PROG = _build_program()
    return _PROG


def _make_in_maps(x, mask, W_attn, W_proj):
    mask00 = np.asarray(mask[0, 0], dtype=np.float32)
    masks = np.stack([np.maximum(mask00[0:TB, m * 128:(m + 1) * 128].T, NEG_CLAMP)
                      for m in range(4)]).astype(np.float32)
    masks = np.ascontiguousarray(masks)
    ident = np.eye(128, dtype=np.float32)
    onesd = np.ones((128, 64), np.float32)
    in_maps = []
    for c in range(8):
        b, g = divmod(c, 2)
        wq = W_attn[:, g * CP:(g + 1) * CP] * np.float32(1.0 / np.sqrt(D))
        wk = W_attn[:, C + g * CP:C + (g + 1) * CP]
        wv = W_attn[:, 2 * C + g * CP:2 * C + (g + 1) * CP]
        in_maps.append({
            "x_s": np.ascontiguousarray(x[b]),
            "wqk": np.ascontiguousarray(np.concatenate([wq, wk], axis=1)),
            "wv": np.ascontiguousarray(wv),
            "wp": np.ascontiguousarray(W_proj[g * CP:(g + 1) * CP, :]),
            "masks": masks,
            "ident": ident,
            "onesd": onesd,
        })
    return in_maps


def _assemble(results, b_attn, b_proj, W_proj):
    out = np.empty((B, T, C), np.float32)
    k = np.empty((B, H, T, D), np.float32)
    v = np.empty((B, H, T, D), np.float32)
    bk = b_attn[C:2 * C]
    bv = b_attn[2 * C:3 * C]
    extra = b_proj.astype(np.float64) + bv.astype(np.float64) @ W_proj.astype(np.float64)
    for c in range(8):
        b, g = divmod(c, 2)
        r = results[c]
        if g == 0:
            out[b] = r["out_p"]
        else:
            out[b] += r["out_p"]
        for h in range(HL):
            gh = g * HL + h
            k[b, gh] = r["k_p"][h] + bk[gh * D:(gh + 1) * D][None, :]
            v[b, gh] = r["v_p"][h] + bv[gh * D:(gh + 1) * D][None, :]
    out += extra.astype(np.float32)[None, None, :]
    return out, k, v


def _reference_numpy(x, mask, W_attn, b_attn, W_proj, b_proj):
    x64 = x.astype(np.float64)
    qkv = x64 @ W_attn.astype(np.float64) + b_attn.astype(np.float64)
    q, kk, vv = np.split(qkv, 3, axis=2)
    q = q.reshape(B, T, H, D).transpose(0, 2, 1, 3)
    kk = kk.reshape(B, T, H, D).transpose(0, 2, 1, 3)
    vv = vv.reshape(B, T, H, D).transpose(0, 2, 1, 3)
    att = np.einsum('bhqd,bhkd->bhqk', q, kk) / np.sqrt(D)
    att = att + mask.astype(np.float64)
    att = att - att.max(axis=-1, keepdims=True)
    att = np.exp(att)
    att /= att.sum(axis=-1, keepdims=True)
    o = np.einsum('bhqk,bhkd->bhqd', att, vv)
    o = o.transpose(0, 2, 1, 3).reshape(B, T, C)
    o = o @ W_proj.astype(np.float64) + b_proj.astype(np.float64)
    return (o.astype(np.float32), kk.astype(np.float32), vv.astype(np.float32))


def kernel(x, mask, W_attn, b_attn, W_proj, b_proj):
    from concourse.bass_utils import run_bass_kernel_spmd
    x = np.asarray(x, np.float32)
    mask = np.asarray(mask, np.float32)
    W_attn = np.asarray(W_attn, np.float32)
    b_attn = np.asarray(b_attn, np.float32)
    W_proj = np.asarray(W_proj, np.float32)
    b_proj = np.asarray(b_proj, np.float32)

    causal = np.array_equal(
        np.asarray(mask[0, 0]),
        np.triu(np.full((T, T), -1e9, dtype=np.float32), k=1))
    if not causal or np.any(b_attn[0:C]):
        return _reference_numpy(x, mask, W_attn, b_attn, W_proj, b_proj)

    prog = get_program()
    in_maps = _make_in_maps(x, mask, W_attn, W_proj)
    res = run_bass_kernel_spmd(prog, in_maps, core_ids=list(range(8)))
    return _assemble(res.results, b_attn, b_proj, W_proj)
